# revision 8
# baseline (speedup 1.0000x reference)
"""Self-contained Trainium2 Bass kernel for causal MHA.

Problem: B=32, L=512, D=4096, H=32 heads (head_dim 128), causal attention,
torch-Linear projections (y = x @ W.T + b).

Strategy: data-parallel over batch across the 8 NeuronCores (4 batches each).
Per core, everything is computed in "transposed activation" layout so no
on-chip transposes are ever needed:
  - Q.T, K.T per head:  [head_dim(part), tok]   (lhsT = W.T tile, rhs = x.T)
  - V natural per head-group: [tok(part), feat]  (lhsT = x.T tile, rhs = Wv.T)
  - S.T = K.T-chunk.T @ Q.T -> [key(part), query]; softmax denominator via an
    all-ones stationary matmul (broadcasts column sums to all partitions);
    causal handled by a multiplicative mask after exp.
  - out.T = V-chunk.T @ p.T accumulated over key chunks -> [head_dim, tok],
    normalized by the reciprocal of the ones-matmul output.
  - y = attout.T-tile.T @ Wo.T chunk (natural layout), + bias, DMA out.
Matmuls run in bf16 (fp32 accumulate in PSUM).
"""

import os
import sys

sys.path.insert(0, "/opt/trn_rl_repo")

import numpy as np
import ml_dtypes

import concourse.bass as bass
import concourse.mybir as mybir
import concourse.tile as tile
from concourse import bacc
from concourse import bass_utils
from concourse.bass_interp import get_hw_module

BF16 = mybir.dt.bfloat16
F32 = mybir.dt.float32
NPBF16 = ml_dtypes.bfloat16
AFT = mybir.ActivationFunctionType

B, L, D, H = 32, 512, 4096, 32
HD = 128
NCORES = 8
BC = B // NCORES          # batches per core
KO = D // 128             # 32 contraction tiles
NCH = D // 512            # 8 output-feature chunks of 512
NG = H // 4               # 8 head groups of 4 heads
SCALE = 1.0 / float(np.sqrt(HD))

_CACHE = {}


def _build():
    nc = bacc.Bacc(
        "TRN2", target_bir_lowering=False, debug=False, enable_asserts=False
    )
    xT = nc.dram_tensor("xT", [KO, BC, 128, 512], BF16, kind="ExternalInput").ap()
    wq = nc.dram_tensor("wq", [KO, H, 128, 128], BF16, kind="ExternalInput").ap()
    wk = nc.dram_tensor("wk", [KO, H, 128, 128], BF16, kind="ExternalInput").ap()
    wv = nc.dram_tensor("wv", [KO, NCH, 128, 512], BF16, kind="ExternalInput").ap()
    wo = nc.dram_tensor("wo", [KO, NCH, 128, 512], BF16, kind="ExternalInput").ap()
    bqr = nc.dram_tensor("bqr", [128, H], F32, kind="ExternalInput").ap()
    bkr = nc.dram_tensor("bkr", [128, H], F32, kind="ExternalInput").ap()
    bvb = nc.dram_tensor("bvb", [128, D], F32, kind="ExternalInput").ap()
    bob = nc.dram_tensor("bob", [128, D], F32, kind="ExternalInput").ap()
    trid = nc.dram_tensor("trid", [128, 128], BF16, kind="ExternalInput").ap()
    onesd = nc.dram_tensor("onesd", [128, 128], BF16, kind="ExternalInput").ap()
    y = nc.dram_tensor("y", [BC * 512, D], F32, kind="ExternalOutput").ap()

    ts = bass.ts

    with tile.TileContext(nc) as tc:
        with tc.tile_pool(name="const", bufs=1) as constp, \
             tc.tile_pool(name="xpool", bufs=1) as xpool, \
             tc.tile_pool(name="wqk", bufs=3) as wqkp, \
             tc.tile_pool(name="wstream", bufs=3) as wsp, \
             tc.tile_pool(name="qk", bufs=5) as qkp, \
             tc.tile_pool(name="vg", bufs=2) as vgp, \
             tc.tile_pool(name="pt", bufs=18) as ptp, \
             tc.tile_pool(name="ptd", bufs=6) as ptdp, \
             tc.tile_pool(name="rr", bufs=2) as rrp, \
             tc.tile_pool(name="att", bufs=1) as attp, \
             tc.tile_pool(name="yout", bufs=2) as youtp, \
             tc.tile_pool(name="psA", bufs=2, space="PSUM") as psA, \
             tc.tile_pool(name="psB", bufs=2, space="PSUM") as psB:

            tri_sb = constp.tile([128, 128], BF16)
            nc.sync.dma_start(tri_sb[:], trid[:])
            ones_sb = constp.tile([128, 128], BF16)
            nc.sync.dma_start(ones_sb[:], onesd[:])
            bq_sb = constp.tile([128, H], F32)
            nc.sync.dma_start(bq_sb[:], bqr[:])
            bk_sb = constp.tile([128, H], F32)
            nc.sync.dma_start(bk_sb[:], bkr[:])
            bv_sb = constp.tile([128, D], F32)
            nc.sync.dma_start(bv_sb[:], bvb[:])
            bo_sb = constp.tile([128, D], F32)
            nc.sync.dma_start(bo_sb[:], bob[:])

            for b in range(BC):
                xT_sb = xpool.tile([128, KO, 512], BF16, tag="xT")
                nc.sync.dma_start(
                    xT_sb[:], xT[:, b].rearrange("ko ki m -> ki ko m")
                )
                attout = attp.tile([128, H, 512], BF16, tag="attout")

                for g in range(NG):
                    # ---- V for the 4 heads of this group: [tok, 512 feats]
                    v_sb = vgp.tile([128, 4, 512], BF16, tag="vg")
                    for mp in range(2):
                        v_ps0 = psB.tile([128, 512], F32, tag="vy")
                        v_ps1 = psB.tile([128, 512], F32, tag="vy")
                        for kb in range(KO // 8):
                            wv_t = wsp.tile([128, 8, 512], BF16, tag="wst")
                            nc.sync.dma_start(
                                wv_t[:],
                                wv[8 * kb:8 * kb + 8, g].rearrange(
                                    "ko ki n -> ki ko n"),
                            )
                            for r in range(8):
                                ko = 8 * kb + r
                                nc.tensor.matmul(
                                    v_ps0[:], xT_sb[:, ko, ts(2 * mp, 128)],
                                    wv_t[:, r, :],
                                    start=(ko == 0), stop=(ko == KO - 1),
                                )
                                nc.tensor.matmul(
                                    v_ps1[:], xT_sb[:, ko, ts(2 * mp + 1, 128)],
                                    wv_t[:, r, :],
                                    start=(ko == 0), stop=(ko == KO - 1),
                                )
                        nc.vector.tensor_add(
                            v_sb[:, 2 * mp, :], v_ps0[:], bv_sb[:, ts(g, 512)]
                        )
                        nc.vector.tensor_add(
                            v_sb[:, 2 * mp + 1, :], v_ps1[:], bv_sb[:, ts(g, 512)]
                        )

                    # ---- Q.T/K.T for all 4 heads first (gives PE runway to
                    # hide the per-head softmax vector work below)
                    qs, ks = [], []
                    for hh in range(4):
                        h = 4 * g + hh
                        wq_sb = wqkp.tile([128, KO, 128], BF16, tag="w")
                        nc.sync.dma_start(
                            wq_sb[:], wq[:, h].rearrange("ko ki n -> ki ko n")
                        )
                        q_ps = psA.tile([128, 512], F32, tag="qk")
                        for ko in range(KO):
                            nc.tensor.matmul(
                                q_ps[:], wq_sb[:, ko, :], xT_sb[:, ko, :],
                                start=(ko == 0), stop=(ko == KO - 1),
                            )
                        q_sb = qkp.tile([128, 512], BF16, tag="q")
                        nc.scalar.activation(
                            q_sb[:], q_ps[:], AFT.Identity,
                            bias=bq_sb[:, h:h + 1],
                        )
                        qs.append(q_sb)

                        wk_sb = wqkp.tile([128, KO, 128], BF16, tag="w")
                        nc.sync.dma_start(
                            wk_sb[:], wk[:, h].rearrange("ko ki n -> ki ko n")
                        )
                        k_ps = psA.tile([128, 512], F32, tag="qk")
                        for ko in range(KO):
                            nc.tensor.matmul(
                                k_ps[:], wk_sb[:, ko, :], xT_sb[:, ko, :],
                                start=(ko == 0), stop=(ko == KO - 1),
                            )
                        k_sb = qkp.tile([128, 512], BF16, tag="k")
                        nc.scalar.activation(
                            k_sb[:], k_ps[:], AFT.Identity,
                            bias=bk_sb[:, h:h + 1],
                        )
                        ks.append(k_sb)

                    # ---- S.T + exp + causal mask for ALL 4 heads first, so
                    # the R/av matmuls of head hh overlap the exp/mask vector
                    # chains of heads hh+1..  Chunk c only contributes to
                    # queries >= 128c (causal): N_c = 512-128c columns.
                    ptss = []
                    for hh in range(4):
                        q_sb, k_sb = qs[hh], ks[hh]
                        pts = []
                        for c in range(4):
                            n_c = 512 - 128 * c
                            st_ps = psA.tile([128, 512], F32, tag="st")
                            nc.tensor.matmul(
                                st_ps[:, 0:n_c], k_sb[:, ts(c, 128)],
                                q_sb[:, 128 * c:512],
                                start=True, stop=True,
                            )
                            pt_c = ptp.tile([128, 512], BF16, tag="pt")
                            if n_c > 128:
                                nc.scalar.activation(
                                    pt_c[:, 128:n_c], st_ps[:, 128:n_c],
                                    AFT.Exp, scale=SCALE,
                                )
                            ptd = ptdp.tile([128, 128], BF16, tag="ptd")
                            nc.scalar.activation(
                                ptd[:], st_ps[:, 0:128], AFT.Exp, scale=SCALE
                            )
                            nc.vector.tensor_mul(
                                pt_c[:, 0:128], ptd[:], tri_sb[:]
                            )
                            pts.append((pt_c, n_c))
                        ptss.append(pts)

                    for hh in range(4):
                        h = 4 * g + hh
                        pts = ptss[hh]
                        # ---- softmax denominator (broadcast to all partitions)
                        r_ps = psA.tile([128, 512], F32, tag="rout")
                        for c in range(4):
                            pt_c, n_c = pts[c]
                            nc.tensor.matmul(
                                r_ps[:, 128 * c:512], ones_sb[:],
                                pt_c[:, 0:n_c],
                                start=(c == 0), stop=(c == 3),
                            )
                        rrec = rrp.tile([128, 512], F32, tag="rr")
                        nc.vector.reciprocal(rrec[:], r_ps[:])

                        # ---- out.T[h] = sum_c V_c.T @ p.T_c, then normalize
                        o_ps = psA.tile([128, 512], F32, tag="rout")
                        for c in range(4):
                            pt_c, n_c = pts[c]
                            nc.tensor.matmul(
                                o_ps[:, 128 * c:512], v_sb[:, c, ts(hh, 128)],
                                pt_c[:, 0:n_c],
                                start=(c == 0), stop=(c == 3),
                            )
                        nc.vector.tensor_mul(
                            attout[:, h, :], o_ps[:], rrec[:]
                        )

                # ---- output projection: y[tok, feat] += bias
                for nc_ in range(NCH):
                    for mp in range(2):
                        y_ps0 = psB.tile([128, 512], F32, tag="vy")
                        y_ps1 = psB.tile([128, 512], F32, tag="vy")
                        for kb in range(KO // 8):
                            wo_t = wsp.tile([128, 8, 512], BF16, tag="wst")
                            nc.sync.dma_start(
                                wo_t[:],
                                wo[8 * kb:8 * kb + 8, nc_].rearrange(
                                    "ko ki n -> ki ko n"),
                            )
                            for r in range(8):
                                ko = 8 * kb + r
                                nc.tensor.matmul(
                                    y_ps0[:], attout[:, ko, ts(2 * mp, 128)],
                                    wo_t[:, r, :],
                                    start=(ko == 0), stop=(ko == KO - 1),
                                )
                                nc.tensor.matmul(
                                    y_ps1[:], attout[:, ko, ts(2 * mp + 1, 128)],
                                    wo_t[:, r, :],
                                    start=(ko == 0), stop=(ko == KO - 1),
                                )
                        for j, y_ps in ((0, y_ps0), (1, y_ps1)):
                            y_sb = youtp.tile([128, 512], F32, tag="y")
                            nc.vector.tensor_add(
                                y_sb[:], y_ps[:], bo_sb[:, ts(nc_, 512)]
                            )
                            m_tile = 2 * mp + j
                            nc.sync.dma_start(
                                y[512 * b + 128 * m_tile:512 * b + 128 * (m_tile + 1),
                                  ts(nc_, 512)],
                                y_sb[:],
                            )

    nc.compile()
    nc.m = get_hw_module(nc.m)
    return nc


def _prep_inputs(x, Wq, bq, Wk, bk, Wv, bv, Wo, bo):
    """Host-side layout prep. Returns the per-core input maps."""
    x = np.asarray(x, dtype=np.float32)
    Wq = np.asarray(Wq, dtype=np.float32)
    Wk = np.asarray(Wk, dtype=np.float32)
    Wv = np.asarray(Wv, dtype=np.float32)
    Wo = np.asarray(Wo, dtype=np.float32)
    bq = np.asarray(bq, dtype=np.float32)
    bk = np.asarray(bk, dtype=np.float32)
    bv = np.asarray(bv, dtype=np.float32)
    bo = np.asarray(bo, dtype=np.float32)

    def lhs_blocks(W):  # [KO, H, 128ki, 128n]
        return np.ascontiguousarray(
            W.reshape(H, 128, KO, 128).transpose(2, 0, 3, 1)
        ).astype(NPBF16)

    def rhs_blocks(W):  # [KO, NCH, 128ki, 512n]
        return np.ascontiguousarray(
            W.reshape(NCH, 512, KO, 128).transpose(2, 0, 3, 1)
        ).astype(NPBF16)

    wq_b = lhs_blocks(Wq)
    wk_b = lhs_blocks(Wk)
    wv_b = rhs_blocks(Wv)
    wo_b = rhs_blocks(Wo)
    bqr = np.ascontiguousarray(bq.reshape(H, 128).T)
    bkr = np.ascontiguousarray(bk.reshape(H, 128).T)
    bvb = np.ascontiguousarray(np.broadcast_to(bv, (128, D)))
    bob = np.ascontiguousarray(np.broadcast_to(bo, (128, D)))

    i = np.arange(128)[:, None]
    j = np.arange(128)[None, :]
    tri = (i <= j).astype(NPBF16)
    ones = np.ones((128, 128), dtype=NPBF16)

    in_maps = []
    for core in range(NCORES):
        xc = x[BC * core:BC * (core + 1)]          # [BC, 512, 4096]
        xT = np.ascontiguousarray(
            xc.reshape(BC, 512, KO, 128).transpose(2, 0, 3, 1)
        ).astype(NPBF16)                           # [KO, BC, 128ki, 512m]
        in_maps.append({
            "xT": xT, "wq": wq_b, "wk": wk_b, "wv": wv_b, "wo": wo_b,
            "bqr": bqr, "bkr": bkr, "bvb": bvb, "bob": bob,
            "trid": tri, "onesd": ones,
        })
    return in_maps


def _get_nc():
    if "nc" not in _CACHE:
        _CACHE["nc"] = _build()
    return _CACHE["nc"]


def run(trace=False, **inputs):
    """Run on the 8 NeuronCores. Returns (y, BassKernelResults)."""
    nc = _get_nc()
    in_maps = _prep_inputs(**inputs)
    res = bass_utils.run_bass_kernel_spmd(
        nc, in_maps, core_ids=list(range(NCORES)), trace=trace
    )
    y = np.stack([res.results[c]["y"] for c in range(NCORES)], axis=0)
    y = y.reshape(B, L, D)
    return y, res


def kernel(**inputs):
    y, _ = run(trace=False, **inputs)
    return y
